# revision 14
# baseline (speedup 1.0000x reference)
"""A3TGCN (2-layer TGCN + temporal attention) distributed Bass kernel for
8 Trainium2 NeuronCores.

Math restructuring (validated vs reference):
  - PyG GCNConv:  gcn(h, Wc, bc) = Ahat @ (h Wc) + bc  where
      Ahat = D^-1/2 (A_w + I) D^-1/2  (self loops appended as edges).
    Associativity: Ahat (h Wc) = (Ahat h) Wc, so the three gates of a TGCN
    cell share ONE sparse aggregation  s = Ahat h.
  - Gate algebra folds to   z = sigmoid(s A_z + h B_z + c_z)  etc, with
      A_g = Wc_g Wl_g[:H],  B_g = Wl_g[H:],  c_g = bc_g Wl_g[:H] + bl_g.
  - Layer-0 aggregation inputs are x_t (no recurrence) and layer-1 inputs
    are layer-0 outputs, so the 32 aggregations batch into TWO passes over
    the edges with T*H = 2048 features each.

v2 (perf): gathered features are fp8 e3m4 (x scaled x2 host-side, h0
scaled x8 on device; the scales are folded out of the per-layer gate A_g
matrices) which halves gather DMA and AllGather bytes. Gather chunks are
GB=6 groups (768 rows) to amortize the ~1us fixed SWDGE descriptor-gen
cost per dma_gather call. GRU0 runs chain0 then three 256-node chains
round-robined with pass-B round-0 chunks woven in so the AG0 window and
the recurrence stalls are filled with aggregation matmuls. GRU1 runs in
three tile-aligned chains; attention + output head are deferred to the
end and use PE ones-matmul broadcasts (no gpsimd partition_broadcast, one
Exp table load).
"""
import numpy as np
import ml_dtypes

import concourse.bass as bass
import concourse.tile as tile
from concourse import bacc, mybir
from concourse.bass_utils import run_bass_kernel_spmd
from concourse.masks import make_identity

# problem constants
N, E, F, H, T, L, O = 10000, 320000, 128, 128, 16, 2, 128
P = 128
N_CORES = 8
N_LOCAL = N // N_CORES            # 1250
N_TILES = (N_LOCAL + P - 1) // P  # 10
N_PAD = N_TILES * P               # 1280
C = T * H                         # 2048 features per aggregation pass
GB = 5                            # groups per gather chunk (640 rows)
SEGS = [(0, 512), (512, 768)]     # (base, width) local-node AG segments
# GRU0 recurrence chains: chain0 feeds AG0; chains 1-2 feed AG1.
# Only TWO chains may round-robin concurrently: the gz/gr PSUM ping-pong
# plus in-order engine queues deadlocks with three interleaved chains.
CHAINS0 = [(0, 512), (512, 384), (896, 384)]
# GRU1 node chunks (A runs alone, B+C run as one two-chunk chain)
CHAINS1 = [(0, 512), (512, 384), (896, 384)]
NSEG = len(SEGS)
SX = 2.8                          # x fp8 scale (|SX*x| < 15.5 e3m4 max)
SH = 15.0                         # h0 fp8 scale (|h0| <= 1)
BF16 = mybir.dt.bfloat16
F32 = mybir.dt.float32
FP8 = mybir.dt.float8e3
I16 = mybir.dt.int16


# ----------------------------------------------------------------- host prep
def _prep_graph(edge_index, edge_weight):
    src = np.asarray(edge_index[0], np.int64)
    dst = np.asarray(edge_index[1], np.int64)
    ew = np.asarray(edge_weight, np.float64)

    deg = np.zeros(N)
    np.add.at(deg, dst, ew)
    deg += 1.0
    dinv = 1.0 / np.sqrt(deg)
    norm_e = (dinv[src] * ew * dinv[dst]).astype(np.float32)
    self_norm = (dinv * dinv).astype(np.float32)
    src_all = np.concatenate([src, np.arange(N)])
    dst_all = np.concatenate([dst, np.arange(N)])
    w_all = np.concatenate([norm_e, self_norm]).astype(np.float32)
    core_of = dst_all // N_LOCAL

    # per (core, tile): sorted unique srcs; per unique: pass-B segment + rank
    info = {}
    GA = np.zeros(N_TILES, np.int64)
    GBseg = np.zeros((NSEG, N_TILES), np.int64)
    for c in range(N_CORES):
        m = core_of == c
        s_, d_, w_ = src_all[m], dst_all[m] - c * N_LOCAL, w_all[m]
        tl = d_ // P
        for u in range(N_TILES):
            mu = tl == u
            su, du, wu = s_[mu], d_[mu] - u * P, w_[mu]
            us, inv = np.unique(su, return_inverse=True)
            locs = us % N_LOCAL
            seg_of = np.zeros(len(us), np.int64)
            rank_of = np.zeros(len(us), np.int64)
            for si, (base, wdt) in enumerate(SEGS):
                sel = (locs >= base) & (locs < base + wdt)
                seg_of[sel] = si
                rank_of[sel] = np.arange(int(sel.sum()))
                GBseg[si, u] = max(GBseg[si, u], (int(sel.sum()) + P - 1) // P)
            GA[u] = max(GA[u], (len(us) + P - 1) // P)
            info[(c, u)] = (us, inv, du, wu, seg_of, rank_of)

    gbaseA = np.concatenate([[0], np.cumsum(GA)])
    sumA = int(GA.sum())
    gbaseB = np.zeros((NSEG, N_TILES), np.int64)
    acc = 0
    for si in range(NSEG):
        for u in range(N_TILES):
            gbaseB[si, u] = acc
            acc += GBseg[si, u]
    sumB = int(acc)

    idxA = np.zeros((N_CORES, 16, sumA * 8), np.int16)
    idxB = np.zeros((N_CORES, 16, sumB * 8), np.int16)
    sgA = np.zeros((N_CORES, P, sumA * P), np.float32)
    sgB = np.zeros((N_CORES, P, sumB * P), np.float32)

    def put_idx(tab, gbase, vals):
        n = len(vals)
        tab[:, gbase * 8: gbase * 8 + n // 16] = vals.reshape(-1, 16).T

    for c in range(N_CORES):
        for u in range(N_TILES):
            us, inv, du, wu, seg_of, rank_of = info[(c, u)]
            # pass A
            vals = np.zeros(int(GA[u]) * P, np.int16)
            vals[:len(us)] = us.astype(np.int16)
            put_idx(idxA[c], int(gbaseA[u]), vals)
            eslot = np.arange(len(us))[inv]
            np.add.at(sgA[c], (eslot % P, (int(gbaseA[u]) + eslot // P) * P + du), wu)
            # pass B per segment
            owner = us // N_LOCAL
            locs = us % N_LOCAL
            for si, (base, wdt) in enumerate(SEGS):
                sel = seg_of == si
                nsl = int(sel.sum())
                g0 = int(gbaseB[si, u])
                vals = np.zeros(int(GBseg[si, u]) * P, np.int16)
                vals[:nsl] = (owner[sel] * wdt + (locs[sel] - base)).astype(np.int16)
                put_idx(idxB[c], g0, vals)
                em = seg_of[inv] == si
                ej = rank_of[inv][em]
                np.add.at(sgB[c], (ej % P, (g0 + ej // P) * P + du[em]), wu[em])

    idxA = np.ascontiguousarray(np.tile(idxA, (1, 8, 1)))
    idxB = np.ascontiguousarray(np.tile(idxB, (1, 8, 1)))
    return (tuple(int(v) for v in GA),
            tuple(tuple(int(v) for v in GBseg[si]) for si in range(NSEG)),
            idxA, idxB,
            sgA.astype(ml_dtypes.bfloat16), sgB.astype(ml_dtypes.bfloat16))


def _fold_weights(inp):
    """Wpack (128, 12*128) bf16, lhsT blocks ordered [l][gate z,r,h][A|B];
    biases (128, 8) f32, col = l*3 + gate. The A_g blocks absorb 1/scale of
    the fp8 feature scaling of that layer's aggregation inputs."""
    W = np.zeros((12, H, H), np.float64)
    Bias = np.zeros((H, 8), np.float64)
    s_scale = {0: SX, 1: SH}
    for l in range(L):
        for gi, g in enumerate("zrh"):
            Wc = np.asarray(inp[f"Wc{g}"][l], np.float64)
            bc = np.asarray(inp[f"bc{g}"][l], np.float64)
            Wl = np.asarray(inp[f"Wl{g}"][l], np.float64)
            bl = np.asarray(inp[f"bl{g}"][l], np.float64)
            W[l * 6 + gi * 2] = (Wc @ Wl[:H]) / s_scale[l]   # A_g (descaled)
            W[l * 6 + gi * 2 + 1] = Wl[H:]                   # B_g
            Bias[:, l * 3 + gi] = bc @ Wl[:H] + bl
    # cols 6/7: doubled h-gate bias, for tanh(x+c) = 2*sigmoid(2x+2c)-1
    Bias[:, 6] = 2.0 * Bias[:, 2]
    Bias[:, 7] = 2.0 * Bias[:, 5]
    Wpack = np.transpose(W, (1, 0, 2)).reshape(H, 12 * H)
    return Wpack.astype(ml_dtypes.bfloat16), Bias.astype(np.float32)


# -------------------------------------------------------------- device build
def _build_program(GA, GBsegs):
    GA = np.asarray(GA)
    GBseg = np.asarray(GBsegs)
    sumA = int(GA.sum())
    sumB = int(GBseg.sum())
    gbaseA = np.concatenate([[0], np.cumsum(GA)])
    gbaseB = np.zeros((NSEG, N_TILES), np.int64)
    acc = 0
    for si in range(NSEG):
        for u in range(N_TILES):
            gbaseB[si, u] = acc
            acc += GBseg[si, u]

    nc = bacc.Bacc("TRN2", target_bir_lowering=False, debug=False, num_devices=N_CORES)

    x_dram = nc.dram_tensor("xsrc", [N, C], FP8, kind="ExternalInput")
    idxa_dram = nc.dram_tensor("idxa", [P, sumA * 8], I16, kind="ExternalInput")
    idxb_dram = nc.dram_tensor("idxb", [P, sumB * 8], I16, kind="ExternalInput")
    sga_dram = nc.dram_tensor("sga", [P, sumA * P], BF16, kind="ExternalInput")
    sgb_dram = nc.dram_tensor("sgb", [P, sumB * P], BF16, kind="ExternalInput")
    w_dram = nc.dram_tensor("wpack", [H, 12 * H], BF16, kind="ExternalInput")
    b_dram = nc.dram_tensor("bias", [H, 8], F32, kind="ExternalInput")
    attw_dram = nc.dram_tensor("attw", [H, 1], BF16, kind="ExternalInput")
    outw_dram = nc.dram_tensor("outw", [H, O], BF16, kind="ExternalInput")
    outb_dram = nc.dram_tensor("outb", [O, 1], F32, kind="ExternalInput")
    out_dram = nc.dram_tensor("out", [N_PAD, O], F32, kind="ExternalOutput")

    h0_loc = [nc.dram_tensor(f"h0_loc{si}", [wdt, C], FP8)
              for si, (base, wdt) in enumerate(SEGS)]
    h0_full = [nc.dram_tensor(f"h0_full{si}", [N_CORES * wdt, C], FP8,
                              addr_space="Shared")
               for si, (base, wdt) in enumerate(SEGS)]

    with tile.TileContext(nc) as tc:
        with (
            tc.tile_pool(name="const", bufs=1) as constp,
            tc.tile_pool(name="big", bufs=1) as bigp,
            tc.tile_pool(name="gat", bufs=3) as gatp,
            tc.tile_pool(name="sgp", bufs=3) as sgp,
            tc.tile_pool(name="work", bufs=2) as workp,
            tc.tile_pool(name="state", bufs=2) as statep,
            tc.tile_pool(name="accp", bufs=1, space="PSUM") as accp,
            tc.tile_pool(name="gatesp", bufs=1, space="PSUM") as gatesp,
            tc.tile_pool(name="tpp", bufs=2, space="PSUM") as tpp,
        ):
            # ---- constants / weights
            id_bf = constp.tile([P, P], BF16, name="id_bf")
            make_identity(nc, id_bf[:])
            id_f32 = constp.tile([P, P], F32, name="id_f32")
            make_identity(nc, id_f32[:])
            ones = constp.tile([16, P], BF16, name="ones")
            nc.gpsimd.memset(ones[:], 1.0)
            # attW embedded at col 15 of a zero [H, 31] strip; sliding windows
            # attwE[:, 15-t : 31-t] give the [H, 16] lhsT whose column t is
            # attW (all other columns zero)
            attwE = constp.tile([H, 31], BF16, name="attwE")
            nc.gpsimd.memset(attwE[:], 0.0)
            wsb = constp.tile([H, 12 * H], BF16, name="wsb")
            nc.sync.dma_start(out=wsb[:], in_=w_dram[:])
            bsb = constp.tile([H, 8], F32, name="bsb")
            nc.sync.dma_start(out=bsb[:], in_=b_dram[:])
            attw_sb = constp.tile([H, 1], BF16, name="attw_sb")
            nc.sync.dma_start(out=attw_sb[:], in_=attw_dram[:])
            nc.vector.tensor_copy(attwE[:, 15:16], attw_sb[:])
            outw_sb = constp.tile([H, O], BF16, name="outw_sb")
            nc.sync.dma_start(out=outw_sb[:], in_=outw_dram[:])
            outb_sb = constp.tile([O, 1], F32, name="outb_sb")
            nc.sync.dma_start(out=outb_sb[:], in_=outb_dram[:])
            hzero = constp.tile([H, 512], BF16, name="hzero")
            nc.gpsimd.memset(hzero[:], 0.0)
            sumM = max(sumA, sumB)
            idxa_sb = workp.tile([P, sumM * 8], I16, tag="idx", bufs=1,
                                 name="idxa_sb")
            nc.sync.dma_start(out=idxa_sb[:, :sumA * 8], in_=idxa_dram[:])
            idxa_sb = idxa_sb[:, :sumA * 8]

            # ---- persistent big buffers
            sfeat = bigp.tile([P, N_TILES * C], BF16, name="sfeat")   # (dst,feat)
            sT = bigp.tile([H, T * N_PAD], BF16, name="sT")           # (feat, t*node)
            # hrows (fp8 h0 rows, dead before AG1 completes) shares the slot
            # with h1all (bf16, written by GRU1 which starts later)
            hrows = bigp.tile([P, N_TILES * C], FP8, name="h0T", tag="hrows")

            def w_ap(l, gate, which):  # lhsT block
                k = l * 6 + gate * 2 + which
                return wsb[:, k * H:(k + 1) * H]

            def bias_ap(l, gate):
                return bsb[:, l * 3 + gate:l * 3 + gate + 1]

            # =================== aggregation =====================
            def agg_tile_chunks(tag, u, g0, g1, idx_sb, sg_dram_, src_dram):
                """Yield after each chunk of <=GB groups (gather + sg load +
                matmuls into acc psum)."""
                acc_t = [accp.tile([P, 512], F32, tag=f"acc{i}",
                                   name=f"acc_{tag}_{u}_{i}") for i in range(4)]
                first = True
                g = g0
                while g < g1:
                    gb = min(GB, g1 - g)
                    sg_t = sgp.tile([P, GB * P], BF16, tag="sg",
                                    name=f"sg_{tag}_{u}_{g}")
                    nc.sync.dma_start(out=sg_t[:, :gb * P],
                                      in_=sg_dram_[:, g * P:(g + gb) * P])
                    gat_t = gatp.tile([P, GB, C], FP8, tag="gat",
                                      name=f"gat_{tag}_{u}_{g}")
                    nc.gpsimd.dma_gather(gat_t[:, :gb, :], src_dram[:],
                                         idx_sb[:, g * 8:(g + gb) * 8],
                                         gb * P, gb * P, C)
                    for k in range(gb):
                        last = (g + k == g1 - 1)
                        for ch in range(4):
                            nc.tensor.matmul(
                                acc_t[ch][:],
                                lhsT=sg_t[:, k * P:(k + 1) * P],
                                rhs=gat_t[:, k, ch * 512:(ch + 1) * 512],
                                start=first, stop=last)
                        first = False
                    g += gb
                    yield acc_t
                return

            def drain_copy(u, acc_t):
                for ch in range(4):
                    nc.vector.tensor_copy(
                        sfeat[:, u * C + ch * 512: u * C + (ch + 1) * 512],
                        acc_t[ch][:])

            def drain_add(u, acc_t):
                # sfeat += psum  (vector tensor_tensor reads PSUM)
                for ch in range(4):
                    dstap = sfeat[:, u * C + ch * 512: u * C + (ch + 1) * 512]
                    nc.vector.tensor_tensor(out=dstap, in0=dstap,
                                            in1=acc_t[ch][:],
                                            op=mybir.AluOpType.add)

            def transpose_tile(tag, u):
                for t_ in range(T):
                    tp = tpp.tile([P, P], BF16, tag="tp", name=f"tp{tag}_{u}_{t_}")
                    nc.tensor.transpose(
                        tp[:], sfeat[:, u * C + t_ * H: u * C + (t_ + 1) * H],
                        id_bf[:])
                    nc.vector.tensor_copy(
                        sT[:, t_ * N_PAD + u * P: t_ * N_PAD + (u + 1) * P], tp[:])

            def agg_tiles_gen(tag, tiles, idx_sb, sg_dram_, src_dram, gbase, G,
                              drain, transpose):
                """Generator: yields once per emitted chunk."""
                for u in tiles:
                    g0, g1 = int(gbase[u]), int(gbase[u]) + int(G[u])
                    if g1 > g0:
                        last_acc = None
                        for acc_t in agg_tile_chunks(tag, u, g0, g1, idx_sb,
                                                     sg_dram_, src_dram):
                            last_acc = acc_t
                            yield
                        drain(u, last_acc)
                    if transpose:
                        transpose_tile(tag, u)

            # =================== GRU generators ============================
            def gru_steps(l, chunks, emit_h, post_t):
                """Generator: yields twice per (t, chunk) — after the z/r
                matmuls, and after the rest of the chunk — so aggregation
                chunks can be interleaved into the PE stream to fill the
                recurrence stalls."""
                hprev, hcol = hzero, 0
                for t_ in range(T):
                    hnew, ncol = emit_h(t_)
                    for ci, (c0, cl) in enumerate(chunks):
                        sT_ap = sT[:, t_ * N_PAD + c0: t_ * N_PAD + c0 + cl]
                        if hprev is hzero:
                            hp_ap = hprev[:, 0:cl]
                        else:
                            hp_ap = hprev[:, hcol + c0: hcol + c0 + cl]
                        zp = gatesp.tile([P, cl], F32, tag="gz", name=f"gz{l}_{t_}_{ci}")
                        nc.tensor.matmul(zp[:], lhsT=w_ap(l, 0, 0), rhs=sT_ap,
                                         start=True, stop=False)
                        nc.tensor.matmul(zp[:], lhsT=w_ap(l, 0, 1), rhs=hp_ap,
                                         start=False, stop=True)
                        rp = gatesp.tile([P, cl], F32, tag="gr", name=f"gr{l}_{t_}_{ci}")
                        nc.tensor.matmul(rp[:], lhsT=w_ap(l, 1, 0), rhs=sT_ap,
                                         start=True, stop=False)
                        nc.tensor.matmul(rp[:], lhsT=w_ap(l, 1, 1), rhs=hp_ap,
                                         start=False, stop=True)
                        yield
                        z_sb = workp.tile([P, cl], BF16, tag="z_sb", name=f"z{l}_{t_}_{ci}")
                        nc.scalar.activation(z_sb[:], zp[:],
                                             mybir.ActivationFunctionType.Sigmoid,
                                             bias=bias_ap(l, 0))
                        r_sb = workp.tile([P, cl], BF16, tag="r_sb", name=f"r{l}_{t_}_{ci}")
                        nc.scalar.activation(r_sb[:], rp[:],
                                             mybir.ActivationFunctionType.Sigmoid,
                                             bias=bias_ap(l, 1))
                        rh = workp.tile([P, cl], BF16, tag="rh", name=f"rh{l}_{t_}_{ci}")
                        nc.vector.tensor_tensor(out=rh[:], in0=r_sb[:], in1=hp_ap,
                                                op=mybir.AluOpType.mult)
                        hp_ = gatesp.tile([P, cl], F32, tag="gz", name=f"gh{l}_{t_}_{ci}")
                        nc.tensor.matmul(hp_[:], lhsT=w_ap(l, 2, 0), rhs=sT_ap,
                                         start=True, stop=False)
                        nc.tensor.matmul(hp_[:], lhsT=w_ap(l, 2, 1), rhs=rh[:],
                                         start=False, stop=True)
                        q32 = workp.tile([P, cl], F32, tag="q32", name=f"q{l}_{t_}_{ci}")
                        nc.scalar.activation(q32[:], hp_[:],
                                             mybir.ActivationFunctionType.Sigmoid,
                                             bias=bsb[:, 6 + l:7 + l], scale=2.0)
                        ht = workp.tile([P, cl], BF16, tag="ht", name=f"ht{l}_{t_}_{ci}")
                        nc.vector.tensor_scalar(out=ht[:], in0=q32[:],
                                                scalar1=2.0, scalar2=-1.0,
                                                op0=mybir.AluOpType.mult,
                                                op1=mybir.AluOpType.add)
                        # h_new = ht + z*(hprev - ht)
                        df = workp.tile([P, cl], BF16, tag="df", name=f"df{l}_{t_}_{ci}")
                        nc.vector.tensor_tensor(out=df[:], in0=hp_ap, in1=ht[:],
                                                op=mybir.AluOpType.subtract)
                        zd = workp.tile([P, cl], BF16, tag="zd", name=f"zd{l}_{t_}_{ci}")
                        nc.vector.tensor_tensor(out=zd[:], in0=z_sb[:], in1=df[:],
                                                op=mybir.AluOpType.mult)
                        nc.vector.tensor_tensor(out=hnew[:, ncol + c0: ncol + c0 + cl],
                                                in0=ht[:], in1=zd[:],
                                                op=mybir.AluOpType.add)
                        yield
                    post_t(t_, hnew, ncol)
                    hprev, hcol = hnew, ncol

            def gru0_chain(ci_):
                base, wdt = CHAINS0[ci_]
                tiles = range(base // P, (base + wdt) // P)

                def emit_h(t_):
                    h = statep.tile([H, wdt], BF16, tag=f"h0s{ci_}",
                                    name=f"h0_{ci_}_{t_}")
                    return h, -base  # so ncol + c0 indexes within [0, wdt)

                def post_t(t_, hnew, ncol):
                    for tile_ in tiles:
                        tp = tpp.tile([P, P], BF16, tag="tp",
                                      name=f"tph_{ci_}_{t_}_{tile_}")
                        nc.tensor.transpose(
                            tp[:], hnew[:, ncol + tile_ * P: ncol + (tile_ + 1) * P],
                            id_bf[:])
                        # store h0*SH as fp8 rows for the pass-B gathers
                        nc.vector.tensor_scalar(
                            out=hrows[:, tile_ * C + t_ * H: tile_ * C + (t_ + 1) * H],
                            in0=tp[:], scalar1=SH, scalar2=None,
                            op0=mybir.AluOpType.mult)

                yield from gru_steps(0, [(base, wdt)], emit_h, post_t)

            def finish_seg(si):
                base, wdt = SEGS[si]
                tiles = range(base // P, (base + wdt) // P)
                for k, tile_ in enumerate(tiles):
                    nc.scalar.dma_start(out=h0_loc[si][k * P:(k + 1) * P, :],
                                        in_=hrows[:, tile_ * C:(tile_ + 1) * C])
                nc.gpsimd.collective_compute(
                    "AllGather", mybir.AluOpType.bypass,
                    replica_groups=[list(range(N_CORES))],
                    ins=[h0_loc[si][:, :]], outs=[h0_full[si][:, :]])

            # h1all shares memory with hrows (hrows' last read is the seg-1
            # h0_loc DMA, which completes before GRU1's first write)
            h1all = bigp.tile([H, T * N_PAD], BF16, name="h1all", tag="hrows")

            def gru1_chain(chunks, att_chunks):
                def emit_h(t_):
                    return h1all, t_ * N_PAD

                def post_t(t_, hnew, ncol):
                    pass

                yield from gru_steps(1, chunks, emit_h, post_t)
                # attention + output head for this chain's node chunks, inline
                # so it robins with (and hides under) the other chain's steps
                for (c0, cl) in att_chunks:
                    yield from attention_out(c0, cl)

            # ============== attention + output head (per chunk) ============
            oT_sb = workp.tile([O, N_PAD], F32, tag="oT", bufs=1, name="oT_sb")

            def attention_out(c0, cl):
                """Generator (yields between stages so it can be robined with
                a still-running GRU1 chain)."""
                # scores sc[t, n] accumulate in one PSUM group: the sliding
                # lhsT window attwE[:, 15-t:31-t] has attW in column t only
                sc = gatesp.tile([16, 512], F32, tag="gz", name=f"sc_{c0}")
                for t_ in range(T):
                    nc.tensor.matmul(
                        sc[:, :cl], lhsT=attwE[:, 15 - t_: 31 - t_],
                        rhs=h1all[:, t_ * N_PAD + c0: t_ * N_PAD + c0 + cl],
                        start=(t_ == 0), stop=(t_ == T - 1))
                e_sb = workp.tile([16, 512], BF16, tag="e_sb", name=f"e_{c0}")
                nc.scalar.activation(e_sb[:, :cl], sc[:, :cl],
                                     mybir.ActivationFunctionType.Exp)
                den = gatesp.tile([1, 512], F32, tag="gr", name=f"den_{c0}")
                nc.tensor.matmul(den[:1, :cl], lhsT=ones[0:16, 0:1],
                                 rhs=e_sb[0:16, :cl], start=True, stop=True)
                deninv = workp.tile([1, 512], BF16, tag="deninv", name=f"di_{c0}")
                with nc.allow_low_precision(reason="softmax denom reciprocal"):
                    nc.vector.reciprocal(deninv[:1, :cl], den[:1, :cl])
                yield
                # ctx_un[h, n] = sum_t e[t, n] * h1[h, t, n]; normalize at end
                ctx32 = workp.tile([H, 512], F32, tag="ctx32", bufs=1,
                                   name=f"ctx32_{c0}")
                for t_ in range(T):
                    # select row t of e (id column as lhsT), then broadcast it
                    # across 128 partitions with a ones-row matmul
                    er = accp.tile([1, 512], F32, tag=f"acc{t_ % 2}",
                                   name=f"er_{c0}_{t_}")
                    nc.tensor.matmul(er[:1, :cl], lhsT=id_bf[0:16, t_:t_ + 1],
                                     rhs=e_sb[0:16, :cl], start=True, stop=True)
                    er0 = workp.tile([1, 512], BF16, tag="er0", name=f"er0_{c0}_{t_}")
                    nc.vector.tensor_copy(er0[:1, :cl], er[:1, :cl])
                    bc = accp.tile([P, 512], F32, tag=f"acc{2 + t_ % 2}",
                                   name=f"bc_{c0}_{t_}")
                    nc.tensor.matmul(bc[:, :cl], lhsT=ones[0:1, 0:P],
                                     rhs=er0[0:1, :cl], start=True, stop=True)
                    h1_ap = h1all[:, t_ * N_PAD + c0: t_ * N_PAD + c0 + cl]
                    if t_ == 0:
                        nc.vector.tensor_tensor(out=ctx32[:, :cl], in0=h1_ap,
                                                in1=bc[:, :cl],
                                                op=mybir.AluOpType.mult)
                    else:
                        tmp = workp.tile([H, 512], BF16, tag="ctmp",
                                         name=f"ctmp_{c0}_{t_}")
                        nc.vector.tensor_tensor(out=tmp[:, :cl], in0=h1_ap,
                                                in1=bc[:, :cl],
                                                op=mybir.AluOpType.mult)
                        nc.vector.tensor_tensor(out=ctx32[:, :cl],
                                                in0=ctx32[:, :cl],
                                                in1=tmp[:, :cl],
                                                op=mybir.AluOpType.add)
                    if t_ % 4 == 3:
                        yield
                # ctx = ctx_un / den (broadcast deninv), cast to bf16
                bcd = accp.tile([P, 512], F32, tag="acc3", name=f"bcd_{c0}")
                nc.tensor.matmul(bcd[:, :cl], lhsT=ones[0:1, 0:P],
                                 rhs=deninv[0:1, :cl], start=True, stop=True)
                ctxb = workp.tile([H, 512], BF16, tag="ctxb", name=f"ctxb_{c0}")
                nc.vector.tensor_tensor(out=ctxb[:, :cl], in0=ctx32[:, :cl],
                                        in1=bcd[:, :cl],
                                        op=mybir.AluOpType.mult)
                op_ = accp.tile([O, 512], F32, tag="acc2", name=f"op_{c0}")
                nc.tensor.matmul(op_[:, :cl], lhsT=outw_sb[:], rhs=ctxb[:, :cl],
                                 start=True, stop=True)
                nc.vector.tensor_scalar(out=oT_sb[:, c0:c0 + cl], in0=op_[:, :cl],
                                        scalar1=outb_sb[:, 0:1], scalar2=None,
                                        op0=mybir.AluOpType.add)

            # ====================== emission schedule ======================
            _SENT = object()

            def adv(its, n):
                """Advance round-robin across live iterators by n total steps."""
                cnt = 0
                while its and cnt < n:
                    for it in list(its):
                        if cnt >= n:
                            break
                        if next(it, _SENT) is _SENT:
                            its.remove(it)
                        else:
                            cnt += 1
                return its

            def weave(agg_gen, gru_its, ratio):
                for _ in agg_gen:
                    adv(gru_its, ratio)

            def drain(gen):
                for _ in gen:
                    pass

            # pass A tiles 0-3 (feeds GRU0 chain 0)
            drain(agg_tiles_gen("a", range(0, 4), idxa_sb, sga_dram, x_dram,
                                gbaseA, GA, drain_copy, True))
            # pass A tiles 4-7 interleaved with GRU0 chain 0 (ratio 2 so the
            # chain completes by tile ~7 and AG0 can trigger early)
            g0 = [gru0_chain(0)]
            weave(agg_tiles_gen("a", range(4, 8), idxa_sb, sga_dram, x_dram,
                                gbaseA, GA, drain_copy, True), g0, 2)
            adv(g0, 1 << 30)
            # AG-0 triggers now; pass-A tiles 8-9 run under its flight time
            finish_seg(0)
            drain(agg_tiles_gen("a", range(8, 10), idxa_sb, sga_dram, x_dram,
                                gbaseA, GA, drain_copy, True))
            # reload the shared idx buffer with the pass-B tables (WAR on the
            # last pass-A gather is tracked by the tile framework)
            idxb_sb = workp.tile([P, sumM * 8], I16, tag="idx", bufs=1,
                                 name="idxb_sb")
            nc.sync.dma_start(out=idxb_sb[:, :sumB * 8], in_=idxb_dram[:])
            idxb_sb = idxb_sb[:, :sumB * 8]
            # GRU0 chains 1-2 round-robin; after a lead (covering AG0 flight
            # time) weave in pass-B round-0 chunks for tiles 0-6 (gathers gate
            # on AG0); tiles 7-9 are reserved to fill the AG1 flight window
            ch12 = [gru0_chain(1), gru0_chain(2)]
            adv(ch12, 12)
            weave(agg_tiles_gen("b0", range(0, 7), idxb_sb, sgb_dram,
                                h0_full[0], gbaseB[0], GBseg[0],
                                drain_copy, False), ch12, 1)
            adv(ch12, 1 << 30)
            finish_seg(1)
            # pass-B round-0 leftovers run while AG1 is in flight
            drain(agg_tiles_gen("b0", range(7, 10), idxb_sb, sgb_dram,
                                h0_full[0], gbaseB[0], GBseg[0],
                                drain_copy, False))
            # pass-B round 1 tiles 0-3 (gathers gate on AG1)
            drain(agg_tiles_gen("b1", range(0, 4), idxb_sb, sgb_dram,
                                h0_full[1], gbaseB[1], GBseg[1],
                                drain_add, True))
            # GRU1 chain A (nodes 0-512) woven with round-1 tiles 4-9; each
            # chain runs its attention/output head inline at its end
            gA = gru1_chain([CHAINS1[0]], [CHAINS1[0]])
            ga = [gA]
            weave(agg_tiles_gen("b1", range(4, 10), idxb_sb, sgb_dram,
                                h0_full[1], gbaseB[1], GBseg[1],
                                drain_add, True), ga, 1)
            # chain B+C (one two-chunk chain over nodes 512-1280) joins A
            gBC = gru1_chain(CHAINS1[1:], CHAINS1[1:])
            adv(ga + [gBC], 1 << 30)

            # transpose oT -> out rows and store
            for tile_ in range(N_TILES):
                tp = tpp.tile([P, P], F32, tag="tp", name=f"ot_{tile_}")
                nc.tensor.transpose(tp[:], oT_sb[:, tile_ * P:(tile_ + 1) * P],
                                    id_f32[:])
                ot = workp.tile([P, P], F32, tag="otsb", name=f"otsb_{tile_}")
                nc.vector.tensor_copy(ot[:], tp[:])
                nc.scalar.dma_start(out=out_dram[tile_ * P:(tile_ + 1) * P, :],
                                    in_=ot[:])

    nc.compile()
    return nc


_CACHE = {}


def _get_program(GA, GBsegs):
    key = (GA, GBsegs)
    if key not in _CACHE:
        _CACHE[key] = _build_program(GA, GBsegs)
    return _CACHE[key]


def make_in_maps(inputs):
    x = np.asarray(inputs["x"], np.float32)
    edge_index = np.asarray(inputs["edge_index"])
    edge_weight = np.asarray(inputs["edge_weight"], np.float32)

    GA, GBsegs, idxA, idxB, sgA, sgB = _prep_graph(edge_index, edge_weight)
    wpack, bias = _fold_weights(inputs)
    xb = np.ascontiguousarray(np.transpose(x, (0, 2, 1)).reshape(N, C))
    xb = np.clip(xb * SX, -15.5, 15.5).astype(ml_dtypes.float8_e3m4)
    attw = np.asarray(inputs["att_W"], np.float32).reshape(H, 1).astype(ml_dtypes.bfloat16)
    outw = np.asarray(inputs["out_W"], np.float32).astype(ml_dtypes.bfloat16)
    outb = np.asarray(inputs["out_b"], np.float32).reshape(O, 1)

    in_maps = [
        {
            "xsrc": xb,
            "idxa": idxA[c],
            "idxb": idxB[c],
            "sga": sgA[c],
            "sgb": sgB[c],
            "wpack": wpack,
            "bias": bias,
            "attw": attw,
            "outw": outw,
            "outb": outb,
        }
        for c in range(N_CORES)
    ]
    return (GA, GBsegs), in_maps


def kernel(**inputs) -> np.ndarray:
    (GA, GBsegs), in_maps = make_in_maps(inputs)
    nc = _get_program(GA, GBsegs)
    res = run_bass_kernel_spmd(nc, in_maps, core_ids=list(range(N_CORES)))
    out = np.concatenate([res.results[c]["out"][:N_LOCAL] for c in range(N_CORES)], axis=0)
    return out.astype(np.float32)


# revision 19
# speedup vs baseline: 1.1357x; 1.1357x over previous
"""A3TGCN (2-layer TGCN + temporal attention) distributed Bass kernel for
8 Trainium2 NeuronCores.

Math restructuring (validated vs reference):
  - PyG GCNConv:  gcn(h, Wc, bc) = Ahat @ (h Wc) + bc  where
      Ahat = D^-1/2 (A_w + I) D^-1/2  (self loops appended as edges).
    Associativity: Ahat (h Wc) = (Ahat h) Wc, so the three gates of a TGCN
    cell share ONE sparse aggregation  s = Ahat h.
  - Gate algebra folds to   z = sigmoid(s A_z + h B_z + c_z)  etc, with
      A_g = Wc_g Wl_g[:H],  B_g = Wl_g[H:],  c_g = bc_g Wl_g[:H] + bl_g.
  - Layer-0 aggregation inputs are x_t (no recurrence) and layer-1 inputs
    are layer-0 outputs, so the 32 aggregations batch into TWO passes over
    the edges with T*H = 2048 features each.

v2 (perf): gathered features are fp8 e3m4 (x scaled x2 host-side, h0
scaled x8 on device; the scales are folded out of the per-layer gate A_g
matrices) which halves gather DMA and AllGather bytes. Gather chunks are
GB=6 groups (768 rows) to amortize the ~1us fixed SWDGE descriptor-gen
cost per dma_gather call. GRU0 runs chain0 then three 256-node chains
round-robined with pass-B round-0 chunks woven in so the AG0 window and
the recurrence stalls are filled with aggregation matmuls. GRU1 runs in
three tile-aligned chains; attention + output head are deferred to the
end and use PE ones-matmul broadcasts (no gpsimd partition_broadcast, one
Exp table load).
"""
import numpy as np
import ml_dtypes

import concourse.bass as bass
import concourse.tile as tile
from concourse import bacc, mybir
from concourse.bass_utils import run_bass_kernel_spmd
from concourse.masks import make_identity

# problem constants
N, E, F, H, T, L, O = 10000, 320000, 128, 128, 16, 2, 128
P = 128
N_CORES = 8
N_LOCAL = N // N_CORES            # 1250
N_TILES = (N_LOCAL + P - 1) // P  # 10
N_PAD = N_TILES * P               # 1280
C = T * H                         # 2048 features per aggregation pass
GB = 5                            # groups per gather chunk (640 rows)
SEGS = [(0, 512), (512, 768)]     # (base, width) local-node AG segments
# GRU0 recurrence chains: chain0 feeds AG0; chains 1-2 feed AG1.
# Only TWO chains may round-robin concurrently: the gz/gr PSUM ping-pong
# plus in-order engine queues deadlocks with three interleaved chains.
CHAINS0 = [(0, 512), (512, 384), (896, 384)]
# GRU1 node chunks: staged chains gated on pass-B tile completion; at most
# two chains are ever round-robined together (three would deadlock on the
# gz/gr PSUM ping-pong)
CHAINS1 = [(0, 512), (512, 512), (1024, 256)]
NSEG = len(SEGS)
SX = 2.8                          # x fp8 scale (|SX*x| < 15.5 e3m4 max)
SH = 15.0                         # h0 fp8 scale (|h0| <= 1)
BF16 = mybir.dt.bfloat16
F32 = mybir.dt.float32
FP8 = mybir.dt.float8e3
I16 = mybir.dt.int16


# ----------------------------------------------------------------- host prep
def _prep_graph(edge_index, edge_weight):
    src = np.asarray(edge_index[0], np.int64)
    dst = np.asarray(edge_index[1], np.int64)
    ew = np.asarray(edge_weight, np.float64)

    deg = np.zeros(N)
    np.add.at(deg, dst, ew)
    deg += 1.0
    dinv = 1.0 / np.sqrt(deg)
    norm_e = (dinv[src] * ew * dinv[dst]).astype(np.float32)
    self_norm = (dinv * dinv).astype(np.float32)
    src_all = np.concatenate([src, np.arange(N)])
    dst_all = np.concatenate([dst, np.arange(N)])
    w_all = np.concatenate([norm_e, self_norm]).astype(np.float32)
    core_of = dst_all // N_LOCAL

    # per (core, tile): sorted unique srcs; per unique: pass-B segment + rank
    info = {}
    GA = np.zeros(N_TILES, np.int64)
    GBseg = np.zeros((NSEG, N_TILES), np.int64)
    for c in range(N_CORES):
        m = core_of == c
        s_, d_, w_ = src_all[m], dst_all[m] - c * N_LOCAL, w_all[m]
        tl = d_ // P
        for u in range(N_TILES):
            mu = tl == u
            su, du, wu = s_[mu], d_[mu] - u * P, w_[mu]
            us, inv = np.unique(su, return_inverse=True)
            locs = us % N_LOCAL
            seg_of = np.zeros(len(us), np.int64)
            rank_of = np.zeros(len(us), np.int64)
            for si, (base, wdt) in enumerate(SEGS):
                sel = (locs >= base) & (locs < base + wdt)
                seg_of[sel] = si
                rank_of[sel] = np.arange(int(sel.sum()))
                GBseg[si, u] = max(GBseg[si, u], (int(sel.sum()) + P - 1) // P)
            GA[u] = max(GA[u], (len(us) + P - 1) // P)
            info[(c, u)] = (us, inv, du, wu, seg_of, rank_of)

    gbaseA = np.concatenate([[0], np.cumsum(GA)])
    sumA = int(GA.sum())
    gbaseB = np.zeros((NSEG, N_TILES), np.int64)
    acc = 0
    for si in range(NSEG):
        for u in range(N_TILES):
            gbaseB[si, u] = acc
            acc += GBseg[si, u]
    sumB = int(acc)

    idxA = np.zeros((N_CORES, 16, sumA * 8), np.int16)
    idxB = np.zeros((N_CORES, 16, sumB * 8), np.int16)
    sgA = np.zeros((N_CORES, P, sumA * P), np.float32)
    sgB = np.zeros((N_CORES, P, sumB * P), np.float32)

    def put_idx(tab, gbase, vals):
        n = len(vals)
        tab[:, gbase * 8: gbase * 8 + n // 16] = vals.reshape(-1, 16).T

    for c in range(N_CORES):
        for u in range(N_TILES):
            us, inv, du, wu, seg_of, rank_of = info[(c, u)]
            # pass A
            vals = np.zeros(int(GA[u]) * P, np.int16)
            vals[:len(us)] = us.astype(np.int16)
            put_idx(idxA[c], int(gbaseA[u]), vals)
            eslot = np.arange(len(us))[inv]
            np.add.at(sgA[c], (eslot % P, (int(gbaseA[u]) + eslot // P) * P + du), wu)
            # pass B per segment
            owner = us // N_LOCAL
            locs = us % N_LOCAL
            for si, (base, wdt) in enumerate(SEGS):
                sel = seg_of == si
                nsl = int(sel.sum())
                g0 = int(gbaseB[si, u])
                vals = np.zeros(int(GBseg[si, u]) * P, np.int16)
                vals[:nsl] = (owner[sel] * wdt + (locs[sel] - base)).astype(np.int16)
                put_idx(idxB[c], g0, vals)
                em = seg_of[inv] == si
                ej = rank_of[inv][em]
                np.add.at(sgB[c], (ej % P, (g0 + ej // P) * P + du[em]), wu[em])

    idxA = np.ascontiguousarray(np.tile(idxA, (1, 8, 1)))
    idxB = np.ascontiguousarray(np.tile(idxB, (1, 8, 1)))
    return (tuple(int(v) for v in GA),
            tuple(tuple(int(v) for v in GBseg[si]) for si in range(NSEG)),
            idxA, idxB,
            sgA.astype(ml_dtypes.bfloat16), sgB.astype(ml_dtypes.bfloat16))


def _fold_weights(inp):
    """Wpack (128, 12*128) bf16, lhsT blocks ordered [l][gate z,r,h][A|B];
    biases (128, 8) f32, col = l*3 + gate. The A_g blocks absorb 1/scale of
    the fp8 feature scaling of that layer's aggregation inputs."""
    W = np.zeros((12, H, H), np.float64)
    Bias = np.zeros((H, 8), np.float64)
    s_scale = {0: SX, 1: SH}
    for l in range(L):
        for gi, g in enumerate("zrh"):
            Wc = np.asarray(inp[f"Wc{g}"][l], np.float64)
            bc = np.asarray(inp[f"bc{g}"][l], np.float64)
            Wl = np.asarray(inp[f"Wl{g}"][l], np.float64)
            bl = np.asarray(inp[f"bl{g}"][l], np.float64)
            W[l * 6 + gi * 2] = (Wc @ Wl[:H]) / s_scale[l]   # A_g (descaled)
            W[l * 6 + gi * 2 + 1] = Wl[H:]                   # B_g
            Bias[:, l * 3 + gi] = bc @ Wl[:H] + bl
    # cols 6/7: doubled h-gate bias, for tanh(x+c) = 2*sigmoid(2x+2c)-1
    Bias[:, 6] = 2.0 * Bias[:, 2]
    Bias[:, 7] = 2.0 * Bias[:, 5]
    Wpack = np.transpose(W, (1, 0, 2)).reshape(H, 12 * H)
    return Wpack.astype(ml_dtypes.bfloat16), Bias.astype(np.float32)


# -------------------------------------------------------------- device build
def _build_program(GA, GBsegs):
    GA = np.asarray(GA)
    GBseg = np.asarray(GBsegs)
    sumA = int(GA.sum())
    sumB = int(GBseg.sum())
    gbaseA = np.concatenate([[0], np.cumsum(GA)])
    gbaseB = np.zeros((NSEG, N_TILES), np.int64)
    acc = 0
    for si in range(NSEG):
        for u in range(N_TILES):
            gbaseB[si, u] = acc
            acc += GBseg[si, u]

    nc = bacc.Bacc("TRN2", target_bir_lowering=False, debug=False, num_devices=N_CORES)

    x_dram = nc.dram_tensor("xsrc", [N, C], FP8, kind="ExternalInput")
    idxa_dram = nc.dram_tensor("idxa", [P, sumA * 8], I16, kind="ExternalInput")
    idxb_dram = nc.dram_tensor("idxb", [P, sumB * 8], I16, kind="ExternalInput")
    sga_dram = nc.dram_tensor("sga", [P, sumA * P], BF16, kind="ExternalInput")
    sgb_dram = nc.dram_tensor("sgb", [P, sumB * P], BF16, kind="ExternalInput")
    w_dram = nc.dram_tensor("wpack", [H, 12 * H], BF16, kind="ExternalInput")
    b_dram = nc.dram_tensor("bias", [H, 8], F32, kind="ExternalInput")
    attw_dram = nc.dram_tensor("attw", [H, 1], BF16, kind="ExternalInput")
    outw_dram = nc.dram_tensor("outw", [H, O], BF16, kind="ExternalInput")
    outb_dram = nc.dram_tensor("outb", [O, 1], F32, kind="ExternalInput")
    out_dram = nc.dram_tensor("out", [N_PAD, O], F32, kind="ExternalOutput")

    h0_loc = [nc.dram_tensor(f"h0_loc{si}", [wdt, C], FP8)
              for si, (base, wdt) in enumerate(SEGS)]
    h0_full = [nc.dram_tensor(f"h0_full{si}", [N_CORES * wdt, C], FP8,
                              addr_space="Shared")
               for si, (base, wdt) in enumerate(SEGS)]

    with tile.TileContext(nc) as tc:
        with (
            tc.tile_pool(name="const", bufs=1) as constp,
            tc.tile_pool(name="big", bufs=1) as bigp,
            tc.tile_pool(name="gat", bufs=3) as gatp,
            tc.tile_pool(name="sgp", bufs=3) as sgp,
            tc.tile_pool(name="work", bufs=2) as workp,
            tc.tile_pool(name="state", bufs=2) as statep,
            tc.tile_pool(name="accp", bufs=1, space="PSUM") as accp,
            tc.tile_pool(name="gatesp", bufs=1, space="PSUM") as gatesp,
            tc.tile_pool(name="tpp", bufs=2, space="PSUM") as tpp,
        ):
            # ---- constants / weights
            id_bf = constp.tile([P, P], BF16, name="id_bf")
            make_identity(nc, id_bf[:])
            id_f32 = constp.tile([P, P], F32, name="id_f32")
            make_identity(nc, id_f32[:])
            ones = constp.tile([16, P], BF16, name="ones")
            nc.gpsimd.memset(ones[:], 1.0)
            # attW embedded at col 15 of a zero [H, 31] strip; sliding windows
            # attwE[:, 15-t : 31-t] give the [H, 16] lhsT whose column t is
            # attW (all other columns zero)
            attwE = constp.tile([H, 31], BF16, name="attwE")
            nc.gpsimd.memset(attwE[:], 0.0)
            wsb = constp.tile([H, 12 * H], BF16, name="wsb")
            nc.sync.dma_start(out=wsb[:], in_=w_dram[:])
            bsb = constp.tile([H, 8], F32, name="bsb")
            nc.sync.dma_start(out=bsb[:], in_=b_dram[:])
            attw_sb = constp.tile([H, 1], BF16, name="attw_sb")
            nc.sync.dma_start(out=attw_sb[:], in_=attw_dram[:])
            nc.vector.tensor_copy(attwE[:, 15:16], attw_sb[:])
            outw_sb = constp.tile([H, O], BF16, name="outw_sb")
            nc.sync.dma_start(out=outw_sb[:], in_=outw_dram[:])
            outb_sb = constp.tile([O, 1], F32, name="outb_sb")
            nc.sync.dma_start(out=outb_sb[:], in_=outb_dram[:])
            hzero = constp.tile([H, 512], BF16, name="hzero")
            nc.gpsimd.memset(hzero[:], 0.0)
            sumM = max(sumA, sumB)
            idxa_sb = workp.tile([P, sumM * 8], I16, tag="idx", bufs=1,
                                 name="idxa_sb")
            nc.sync.dma_start(out=idxa_sb[:, :sumA * 8], in_=idxa_dram[:])
            idxa_sb = idxa_sb[:, :sumA * 8]

            # ---- persistent big buffers
            sfeat = bigp.tile([P, N_TILES * C], BF16, name="sfeat")   # (dst,feat)
            sT = bigp.tile([H, T * N_PAD], BF16, name="sT")           # (feat, t*node)
            # hrows (fp8 h0 rows, dead before AG1 completes) shares the slot
            # with h1all (bf16, written by GRU1 which starts later)
            hrows = bigp.tile([P, N_TILES * C], FP8, name="h0T", tag="hrows")

            def w_ap(l, gate, which):  # lhsT block
                k = l * 6 + gate * 2 + which
                return wsb[:, k * H:(k + 1) * H]

            def bias_ap(l, gate):
                return bsb[:, l * 3 + gate:l * 3 + gate + 1]

            # =================== aggregation =====================
            def agg_tile_chunks(tag, u, g0, g1, idx_sb, sg_dram_, src_dram):
                """Yield after each chunk of <=GB groups (gather + sg load +
                matmuls into acc psum)."""
                acc_t = [accp.tile([P, 512], F32, tag=f"acc{i}",
                                   name=f"acc_{tag}_{u}_{i}") for i in range(4)]
                first = True
                g = g0
                while g < g1:
                    gb = min(GB, g1 - g)
                    sg_t = sgp.tile([P, GB * P], BF16, tag="sg",
                                    name=f"sg_{tag}_{u}_{g}")
                    nc.sync.dma_start(out=sg_t[:, :gb * P],
                                      in_=sg_dram_[:, g * P:(g + gb) * P])
                    gat_t = gatp.tile([P, GB, C], FP8, tag="gat",
                                      name=f"gat_{tag}_{u}_{g}")
                    nc.gpsimd.dma_gather(gat_t[:, :gb, :], src_dram[:],
                                         idx_sb[:, g * 8:(g + gb) * 8],
                                         gb * P, gb * P, C)
                    for k in range(gb):
                        last = (g + k == g1 - 1)
                        for ch in range(4):
                            nc.tensor.matmul(
                                acc_t[ch][:],
                                lhsT=sg_t[:, k * P:(k + 1) * P],
                                rhs=gat_t[:, k, ch * 512:(ch + 1) * 512],
                                start=first, stop=last)
                        first = False
                    g += gb
                    yield acc_t
                return

            def drain_copy(u, acc_t):
                for ch in range(4):
                    nc.vector.tensor_copy(
                        sfeat[:, u * C + ch * 512: u * C + (ch + 1) * 512],
                        acc_t[ch][:])

            def drain_add(u, acc_t):
                # sfeat += psum  (vector tensor_tensor reads PSUM)
                for ch in range(4):
                    dstap = sfeat[:, u * C + ch * 512: u * C + (ch + 1) * 512]
                    nc.vector.tensor_tensor(out=dstap, in0=dstap,
                                            in1=acc_t[ch][:],
                                            op=mybir.AluOpType.add)

            def transpose_tile(tag, u):
                for t_ in range(T):
                    tp = tpp.tile([P, P], BF16, tag="tp", name=f"tp{tag}_{u}_{t_}")
                    nc.tensor.transpose(
                        tp[:], sfeat[:, u * C + t_ * H: u * C + (t_ + 1) * H],
                        id_bf[:])
                    nc.vector.tensor_copy(
                        sT[:, t_ * N_PAD + u * P: t_ * N_PAD + (u + 1) * P], tp[:])

            def agg_tiles_gen(tag, tiles, idx_sb, sg_dram_, src_dram, gbase, G,
                              drain, transpose):
                """Generator: yields once per emitted chunk."""
                for u in tiles:
                    g0, g1 = int(gbase[u]), int(gbase[u]) + int(G[u])
                    if g1 > g0:
                        last_acc = None
                        for acc_t in agg_tile_chunks(tag, u, g0, g1, idx_sb,
                                                     sg_dram_, src_dram):
                            last_acc = acc_t
                            yield
                        drain(u, last_acc)
                    if transpose:
                        transpose_tile(tag, u)

            # =================== GRU generators ============================
            def gru_steps(l, chunks, emit_h, post_t):
                """Generator: yields twice per (t, chunk) — after the z/r
                matmuls, and after the rest of the chunk — so aggregation
                chunks can be interleaved into the PE stream to fill the
                recurrence stalls."""
                hprev, hcol = hzero, 0
                for t_ in range(T):
                    hnew, ncol = emit_h(t_)
                    for ci, (c0, cl) in enumerate(chunks):
                        sT_ap = sT[:, t_ * N_PAD + c0: t_ * N_PAD + c0 + cl]
                        if hprev is hzero:
                            hp_ap = hprev[:, 0:cl]
                        else:
                            hp_ap = hprev[:, hcol + c0: hcol + c0 + cl]
                        zp = gatesp.tile([P, cl], F32, tag="gz", name=f"gz{l}_{t_}_{ci}")
                        nc.tensor.matmul(zp[:], lhsT=w_ap(l, 0, 0), rhs=sT_ap,
                                         start=True, stop=False)
                        nc.tensor.matmul(zp[:], lhsT=w_ap(l, 0, 1), rhs=hp_ap,
                                         start=False, stop=True)
                        rp = gatesp.tile([P, cl], F32, tag="gr", name=f"gr{l}_{t_}_{ci}")
                        nc.tensor.matmul(rp[:], lhsT=w_ap(l, 1, 0), rhs=sT_ap,
                                         start=True, stop=False)
                        nc.tensor.matmul(rp[:], lhsT=w_ap(l, 1, 1), rhs=hp_ap,
                                         start=False, stop=True)
                        yield
                        z_sb = workp.tile([P, cl], BF16, tag="z_sb", name=f"z{l}_{t_}_{ci}")
                        nc.scalar.activation(z_sb[:], zp[:],
                                             mybir.ActivationFunctionType.Sigmoid,
                                             bias=bias_ap(l, 0))
                        r_sb = workp.tile([P, cl], BF16, tag="r_sb", name=f"r{l}_{t_}_{ci}")
                        nc.scalar.activation(r_sb[:], rp[:],
                                             mybir.ActivationFunctionType.Sigmoid,
                                             bias=bias_ap(l, 1))
                        rh = workp.tile([P, cl], BF16, tag="rh", name=f"rh{l}_{t_}_{ci}")
                        nc.vector.tensor_tensor(out=rh[:], in0=r_sb[:], in1=hp_ap,
                                                op=mybir.AluOpType.mult)
                        hp_ = gatesp.tile([P, cl], F32, tag="gz", name=f"gh{l}_{t_}_{ci}")
                        nc.tensor.matmul(hp_[:], lhsT=w_ap(l, 2, 0), rhs=sT_ap,
                                         start=True, stop=False)
                        nc.tensor.matmul(hp_[:], lhsT=w_ap(l, 2, 1), rhs=rh[:],
                                         start=False, stop=True)
                        q32 = workp.tile([P, cl], F32, tag="q32", name=f"q{l}_{t_}_{ci}")
                        nc.scalar.activation(q32[:], hp_[:],
                                             mybir.ActivationFunctionType.Sigmoid,
                                             bias=bsb[:, 6 + l:7 + l], scale=2.0)
                        ht = workp.tile([P, cl], BF16, tag="ht", name=f"ht{l}_{t_}_{ci}")
                        nc.vector.tensor_scalar(out=ht[:], in0=q32[:],
                                                scalar1=2.0, scalar2=-1.0,
                                                op0=mybir.AluOpType.mult,
                                                op1=mybir.AluOpType.add)
                        # h_new = ht + z*(hprev - ht)
                        df = workp.tile([P, cl], BF16, tag="df", name=f"df{l}_{t_}_{ci}")
                        nc.vector.tensor_tensor(out=df[:], in0=hp_ap, in1=ht[:],
                                                op=mybir.AluOpType.subtract)
                        zd = workp.tile([P, cl], BF16, tag="zd", name=f"zd{l}_{t_}_{ci}")
                        nc.vector.tensor_tensor(out=zd[:], in0=z_sb[:], in1=df[:],
                                                op=mybir.AluOpType.mult)
                        nc.vector.tensor_tensor(out=hnew[:, ncol + c0: ncol + c0 + cl],
                                                in0=ht[:], in1=zd[:],
                                                op=mybir.AluOpType.add)
                        yield
                    post_t(t_, hnew, ncol)
                    hprev, hcol = hnew, ncol

            def gru0_chain(ci_):
                base, wdt = CHAINS0[ci_]
                tiles = range(base // P, (base + wdt) // P)

                def emit_h(t_):
                    h = statep.tile([H, wdt], BF16, tag=f"h0s{ci_}",
                                    name=f"h0_{ci_}_{t_}")
                    return h, -base  # so ncol + c0 indexes within [0, wdt)

                def post_t(t_, hnew, ncol):
                    for tile_ in tiles:
                        tp = tpp.tile([P, P], BF16, tag="tp",
                                      name=f"tph_{ci_}_{t_}_{tile_}")
                        nc.tensor.transpose(
                            tp[:], hnew[:, ncol + tile_ * P: ncol + (tile_ + 1) * P],
                            id_bf[:])
                        # store h0*SH as fp8 rows for the pass-B gathers
                        nc.vector.tensor_scalar(
                            out=hrows[:, tile_ * C + t_ * H: tile_ * C + (t_ + 1) * H],
                            in0=tp[:], scalar1=SH, scalar2=None,
                            op0=mybir.AluOpType.mult)

                yield from gru_steps(0, [(base, wdt)], emit_h, post_t)

            def finish_seg(si):
                base, wdt = SEGS[si]
                tiles = range(base // P, (base + wdt) // P)
                for k, tile_ in enumerate(tiles):
                    nc.scalar.dma_start(out=h0_loc[si][k * P:(k + 1) * P, :],
                                        in_=hrows[:, tile_ * C:(tile_ + 1) * C])
                nc.gpsimd.collective_compute(
                    "AllGather", mybir.AluOpType.bypass,
                    replica_groups=[list(range(N_CORES))],
                    ins=[h0_loc[si][:, :]], outs=[h0_full[si][:, :]])

            # h1all shares memory with hrows (hrows' last read is the seg-1
            # h0_loc DMA, which completes before GRU1's first write)
            h1all = bigp.tile([H, T * N_PAD], BF16, name="h1all", tag="hrows")

            def gru1_chain(chunks, att_chunks):
                def emit_h(t_):
                    return h1all, t_ * N_PAD

                def post_t(t_, hnew, ncol):
                    pass

                yield from gru_steps(1, chunks, emit_h, post_t)
                # attention + output head for this chain's node chunks, inline
                # so it robins with (and hides under) the other chain's steps
                for (c0, cl) in att_chunks:
                    yield from attention_out(c0, cl)

            # ============== attention + output head (per chunk) ============
            oT_sb = workp.tile([O, N_PAD], F32, tag="oT", bufs=1, name="oT_sb")

            def attention_out(c0, cl):
                """Generator (yields between stages so it can be robined with
                a still-running GRU1 chain)."""
                # scores sc[t, n] accumulate in one PSUM group: the sliding
                # lhsT window attwE[:, 15-t:31-t] has attW in column t only
                sc = tpp.tile([16, 512], F32, tag="tp", name=f"sc_{c0}")
                for t_ in range(T):
                    nc.tensor.matmul(
                        sc[:, :cl], lhsT=attwE[:, 15 - t_: 31 - t_],
                        rhs=h1all[:, t_ * N_PAD + c0: t_ * N_PAD + c0 + cl],
                        start=(t_ == 0), stop=(t_ == T - 1))
                e_sb = workp.tile([16, 512], BF16, tag="e_sb", name=f"e_{c0}")
                nc.scalar.activation(e_sb[:, :cl], sc[:, :cl],
                                     mybir.ActivationFunctionType.Exp)
                den = tpp.tile([1, 512], F32, tag="tp", name=f"den_{c0}")
                nc.tensor.matmul(den[:1, :cl], lhsT=ones[0:16, 0:1],
                                 rhs=e_sb[0:16, :cl], start=True, stop=True)
                deninv = workp.tile([1, 512], BF16, tag="deninv", name=f"di_{c0}")
                with nc.allow_low_precision(reason="softmax denom reciprocal"):
                    nc.vector.reciprocal(deninv[:1, :cl], den[:1, :cl])
                yield
                # ctx_un[h, n] = sum_t e[t, n] * h1[h, t, n]; normalize at end
                ctx32 = workp.tile([H, 512], F32, tag="ctx32", bufs=1,
                                   name=f"ctx32_{c0}")
                for t_ in range(T):
                    # select row t of e (id column as lhsT), then broadcast it
                    # across 128 partitions with a ones-row matmul
                    er = tpp.tile([1, 512], F32, tag="tp", name=f"er_{c0}_{t_}")
                    nc.tensor.matmul(er[:1, :cl], lhsT=id_bf[0:16, t_:t_ + 1],
                                     rhs=e_sb[0:16, :cl], start=True, stop=True)
                    er0 = workp.tile([1, 512], BF16, tag="er0", name=f"er0_{c0}_{t_}")
                    nc.vector.tensor_copy(er0[:1, :cl], er[:1, :cl])
                    bc = tpp.tile([P, 512], F32, tag="tp", name=f"bc_{c0}_{t_}")
                    nc.tensor.matmul(bc[:, :cl], lhsT=ones[0:1, 0:P],
                                     rhs=er0[0:1, :cl], start=True, stop=True)
                    h1_ap = h1all[:, t_ * N_PAD + c0: t_ * N_PAD + c0 + cl]
                    if t_ == 0:
                        nc.vector.tensor_tensor(out=ctx32[:, :cl], in0=h1_ap,
                                                in1=bc[:, :cl],
                                                op=mybir.AluOpType.mult)
                    else:
                        tmp = workp.tile([H, 512], BF16, tag="ctmp",
                                         name=f"ctmp_{c0}_{t_}")
                        nc.vector.tensor_tensor(out=tmp[:, :cl], in0=h1_ap,
                                                in1=bc[:, :cl],
                                                op=mybir.AluOpType.mult)
                        nc.vector.tensor_tensor(out=ctx32[:, :cl],
                                                in0=ctx32[:, :cl],
                                                in1=tmp[:, :cl],
                                                op=mybir.AluOpType.add)
                    if t_ % 4 == 3:
                        yield
                # ctx = ctx_un / den (broadcast deninv), cast to bf16
                bcd = tpp.tile([P, 512], F32, tag="tp", name=f"bcd_{c0}")
                nc.tensor.matmul(bcd[:, :cl], lhsT=ones[0:1, 0:P],
                                 rhs=deninv[0:1, :cl], start=True, stop=True)
                ctxb = workp.tile([H, 512], BF16, tag="ctxb", name=f"ctxb_{c0}")
                nc.vector.tensor_tensor(out=ctxb[:, :cl], in0=ctx32[:, :cl],
                                        in1=bcd[:, :cl],
                                        op=mybir.AluOpType.mult)
                op_ = tpp.tile([O, 512], F32, tag="tp", name=f"op_{c0}")
                nc.tensor.matmul(op_[:, :cl], lhsT=outw_sb[:], rhs=ctxb[:, :cl],
                                 start=True, stop=True)
                nc.vector.tensor_scalar(out=oT_sb[:, c0:c0 + cl], in0=op_[:, :cl],
                                        scalar1=outb_sb[:, 0:1], scalar2=None,
                                        op0=mybir.AluOpType.add)

            # ====================== emission schedule ======================
            _SENT = object()

            def adv(its, n):
                """Advance round-robin across live iterators by n total steps.
                The rotation point persists in the list order across calls so
                interleaving stays a strict alternation (required: irregular
                chain interleavings deadlock on the gz/gr slot ping-pong)."""
                cnt = 0
                while its and cnt < n:
                    it = its.pop(0)
                    if next(it, _SENT) is _SENT:
                        continue
                    its.append(it)
                    cnt += 1
                return its

            def weave(agg_gen, gru_its, ratio):
                # fractional ratio via accumulator: k-th chunk advances
                # int((k+1)*ratio) - int(k*ratio) gru yields
                acc_r = 0.0
                for _ in agg_gen:
                    acc_r += ratio
                    n = int(acc_r)
                    if n:
                        adv(gru_its, n)
                        acc_r -= n

            def adv_until(done_it, its):
                """Advance its round-robin one yield each until done_it is
                exhausted (removed from its)."""
                while done_it in its:
                    adv(its, len(its))

            def drain(gen):
                for _ in gen:
                    pass

            # pass A tiles 0-3 (feeds GRU0 chain 0)
            drain(agg_tiles_gen("a", range(0, 4), idxa_sb, sga_dram, x_dram,
                                gbaseA, GA, drain_copy, True))
            # pass A tiles 4-7 interleaved with GRU0 chain 0 (ratio 2 so the
            # chain completes by tile ~7 and AG0 can trigger early)
            g0 = [gru0_chain(0)]
            weave(agg_tiles_gen("a", range(4, 8), idxa_sb, sga_dram, x_dram,
                                gbaseA, GA, drain_copy, True), g0, 2)
            adv(g0, 1 << 30)
            # AG-0 triggers now; pass-A tiles 8-9 run under its flight time
            finish_seg(0)
            drain(agg_tiles_gen("a", range(8, 10), idxa_sb, sga_dram, x_dram,
                                gbaseA, GA, drain_copy, True))
            # reload the shared idx buffer with the pass-B tables (WAR on the
            # last pass-A gather is tracked by the tile framework)
            idxb_sb = workp.tile([P, sumM * 8], I16, tag="idx", bufs=1,
                                 name="idxb_sb")
            nc.sync.dma_start(out=idxb_sb[:, :sumB * 8], in_=idxb_dram[:])
            idxb_sb = idxb_sb[:, :sumB * 8]
            # GRU0 chains 1-2 round-robin; after a lead (covering AG0 flight
            # time) weave in pass-B round-0 chunks, paced so the chains and
            # the round-0 sweep finish together
            ch12 = [gru0_chain(1), gru0_chain(2)]
            adv(ch12, 12)
            nb0 = sum((int(g) + GB - 1) // GB for g in GBseg[0])
            weave(agg_tiles_gen("b0", range(0, 10), idxb_sb, sgb_dram,
                                h0_full[0], gbaseB[0], GBseg[0],
                                drain_copy, False), ch12, max(1.0, 52.0 / nb0))
            adv(ch12, 1 << 30)
            finish_seg(1)
            # pass-B round 1 tiles 0-3 (gathers gate on AG1)
            drain(agg_tiles_gen("b1", range(0, 4), idxb_sb, sgb_dram,
                                h0_full[1], gbaseB[1], GBseg[1],
                                drain_add, True))
            # GRU1 staged chains, each gated on its tiles' round-1 completion;
            # at most two robined at once; attention/output heads run inline
            # at each chain's end so they hide under the next chain's steps
            gA = gru1_chain([CHAINS1[0]], [CHAINS1[0]])
            ga = [gA]
            weave(agg_tiles_gen("b1", range(4, 8), idxb_sb, sgb_dram,
                                h0_full[1], gbaseB[1], GBseg[1],
                                drain_add, True), ga, 1)
            gB = gru1_chain([CHAINS1[1]], [CHAINS1[1]])
            gab = ga + [gB]
            weave(agg_tiles_gen("b1", range(8, 10), idxb_sb, sgb_dram,
                                h0_full[1], gbaseB[1], GBseg[1],
                                drain_add, True), gab, 1)
            adv_until(gA, gab)
            gC = gru1_chain([CHAINS1[2]], [CHAINS1[2]])
            adv(gab + [gC], 1 << 30)

            # transpose oT -> out rows and store
            for tile_ in range(N_TILES):
                tp = tpp.tile([P, P], F32, tag="tp", name=f"ot_{tile_}")
                nc.tensor.transpose(tp[:], oT_sb[:, tile_ * P:(tile_ + 1) * P],
                                    id_f32[:])
                ot = workp.tile([P, P], F32, tag="otsb", name=f"otsb_{tile_}")
                nc.vector.tensor_copy(ot[:], tp[:])
                nc.scalar.dma_start(out=out_dram[tile_ * P:(tile_ + 1) * P, :],
                                    in_=ot[:])

    nc.compile()
    return nc


_CACHE = {}


def _get_program(GA, GBsegs):
    key = (GA, GBsegs)
    if key not in _CACHE:
        _CACHE[key] = _build_program(GA, GBsegs)
    return _CACHE[key]


def make_in_maps(inputs):
    x = np.asarray(inputs["x"], np.float32)
    edge_index = np.asarray(inputs["edge_index"])
    edge_weight = np.asarray(inputs["edge_weight"], np.float32)

    GA, GBsegs, idxA, idxB, sgA, sgB = _prep_graph(edge_index, edge_weight)
    wpack, bias = _fold_weights(inputs)
    xb = np.ascontiguousarray(np.transpose(x, (0, 2, 1)).reshape(N, C))
    xb = np.clip(xb * SX, -15.5, 15.5).astype(ml_dtypes.float8_e3m4)
    attw = np.asarray(inputs["att_W"], np.float32).reshape(H, 1).astype(ml_dtypes.bfloat16)
    outw = np.asarray(inputs["out_W"], np.float32).astype(ml_dtypes.bfloat16)
    outb = np.asarray(inputs["out_b"], np.float32).reshape(O, 1)

    in_maps = [
        {
            "xsrc": xb,
            "idxa": idxA[c],
            "idxb": idxB[c],
            "sga": sgA[c],
            "sgb": sgB[c],
            "wpack": wpack,
            "bias": bias,
            "attw": attw,
            "outw": outw,
            "outb": outb,
        }
        for c in range(N_CORES)
    ]
    return (GA, GBsegs), in_maps


def kernel(**inputs) -> np.ndarray:
    (GA, GBsegs), in_maps = make_in_maps(inputs)
    nc = _get_program(GA, GBsegs)
    res = run_bass_kernel_spmd(nc, in_maps, core_ids=list(range(N_CORES)))
    out = np.concatenate([res.results[c]["out"][:N_LOCAL] for c in range(N_CORES)], axis=0)
    return out.astype(np.float32)
